# revision 25
# baseline (speedup 1.0000x reference)
"""BatchHardTripletLoss on 8 Trainium2 NeuronCores (Bass/Tile).

Math: for the n x n squared-distance matrix d2[i,j] = sq_i + sq_j - 2*f_i.f_j,
hardest positive = max_{id_j==id_i} dist, hardest negative = min_{id_j!=id_i},
loss = mean(relu(margin + pos - neg)).  Both extremes commute with sqrt/+sq_i,
so each core reduces P[i,j] = delta_j - 2*G[i,j] + BIG*same[i,j] where
delta_j = sq_j - S0, then the host adds sq_i + S0 back.  The BIG*same and
delta_j terms ride a single K=128 one-hot matmul accumulated on top of the
Gram matmul (BIG = 64*64), so no elementwise masking pass is ever needed.

Sharding: rows are sorted by identity on the host; core k owns sorted rows
[k*1024,(k+1)*1024).  Each core receives the full feature matrix rotated so
its own rows sit at local columns [256,1280) - identity groups are contiguous
after the sort, so every same-id entry of every local row lives in local
columns [0,2048) and the hardest-positive max only scans that window.

All O(n*d) prep (transpose, norms, one-hot operands, bf16 casts) happens on
the host; the device does only DMA-in, the O(n^2) matmuls + reductions, and
DMA-out of per-(partition,row-block) accumulators.  Final sqrt/relu/mean is
host fp32.

Per row block rb (128 rows), columns in 8 chunks of 1024:
  - chunks 0,1 (window): DVE takes the fp32 pos-window max straight off
    psum, ACT drains to bf16 for the hardest-neg min tree.
  - chunks 2..6: ACT drains to bf16; DVE runs the pairwise-min tree at 2x.
  - chunk 7: DVE exact min reduce straight off psum.
(GpSimd can neither touch PSUM nor run tensor_tensor min, so the min path
is split ACT/DVE only.)  Window work for all rbs is emitted first so the PE
can start as soon as the first quarter of the columns has landed.
"""

import numpy as np
import ml_dtypes

BF16 = ml_dtypes.bfloat16

N = 8192
D = 128
NCORES = 8
RPC = N // NCORES  # rows per core
RB = RPC // 128  # row blocks per core
MCH = 1024  # psum chunk (2 banks)
NMC = N // MCH
ROW0 = 256  # local column offset of a core's own rows
BIG = 4096.0
S0 = 128.0
MARGIN = 0.2
NID = 64

_cache = {}


def _build_nc(wide):
    from contextlib import ExitStack

    import concourse.bass as bass
    import concourse.bacc as bacc
    import concourse.mybir as mybir
    import concourse.tile as tile

    f32 = mybir.dt.float32
    bf16 = mybir.dt.bfloat16
    AX = mybir.AxisListType.X
    Alu = mybir.AluOpType

    nc = bacc.Bacc(trn_type="TRN2", target_bir_lowering=False, debug=False)
    # device inputs, all host-prepared (bf16, laid out partition-major so
    # every partition's data is one contiguous DRAM run)
    ftb_d = nc.dram_tensor("ftb", [128, N], bf16, kind="ExternalInput")  # F^T
    x_d = nc.dram_tensor("xmat", [128, N], bf16, kind="ExternalInput")  # extras moving
    ftm2_d = nc.dram_tensor("ftm2", [128, RPC], bf16, kind="ExternalInput")  # -2 F^T own
    xl_d = nc.dram_tensor("xl", [128, RPC], bf16, kind="ExternalInput")  # extras lhsT
    negout = nc.dram_tensor("negout", [128, RB * 2], f32, kind="ExternalOutput")
    posout = nc.dram_tensor("posout", [128, RB], f32, kind="ExternalOutput")

    with ExitStack() as ctx:
        tc = ctx.enter_context(tile.TileContext(nc))
        singles = ctx.enter_context(tc.tile_pool(name="singles", bufs=1))
        psum = ctx.enter_context(tc.tile_pool(name="psum", bufs=2, space="PSUM"))
        bwpool = ctx.enter_context(tc.tile_pool(name="bwpool", bufs=3))
        bfpool = ctx.enter_context(tc.tile_pool(name="bfpool", bufs=3))
        b6pool = ctx.enter_context(tc.tile_pool(name="b6pool", bufs=3))
        fpool = ctx.enter_context(tc.tile_pool(name="fpool", bufs=4))
        tpool = ctx.enter_context(tc.tile_pool(name="tpool", bufs=6))
        twpool = ctx.enter_context(tc.tile_pool(name="twpool", bufs=RB))

        ftb = singles.tile([128, N], bf16)
        X = singles.tile([128, N], bf16)
        ftm2 = singles.tile([128, RPC], bf16)
        XL = singles.tile([128, RPC], bf16)
        negacc = singles.tile([128, RB, 2], f32)
        posacc = singles.tile([128, RB], f32)

        # ---- DMAs: the Gram stream (ftb) gets the sync HW queue to itself
        # so matmuls never wait behind X pieces; X rides the gpsimd queue,
        # which demand-paces against the extras matmuls without hurting ----
        nc.gpsimd.dma_start(ftm2, ftm2_d.ap())
        nc.gpsimd.dma_start(XL, xl_d.ap())
        for g in range(8):
            cs = slice(g * 1024, (g + 1) * 1024)
            nc.sync.dma_start(ftb[:, cs], ftb_d.ap()[:, cs])
            nc.gpsimd.dma_start(X[:, cs], x_d.ap()[:, cs])

        # ---- PE warm-up: a few dummy matmuls gated only on the tiny first
        # DMA, so the HAM clock-gate is at 8/8 when the real stream starts ----
        Pwarm = psum.tile([128, 2048], f32, tag="P")
        for i in range(6):
            nc.tensor.matmul(
                Pwarm[:, :256], ftm2[:, 0:128], ftm2[:, 0:256],
                start=True, stop=True,
            )

        def mm_pair(P, rb, c0):
            # two 1024-col chunks into one [128,2048] psum tile, grouped by
            # stationary operand so LDWEIGHTS amortizes
            rs = slice(rb * 128, (rb + 1) * 128)
            for s in range(4):
                col = c0 * MCH + s * 512
                nc.tensor.matmul(
                    P[:, s * 512 : (s + 1) * 512],
                    ftm2[:, rs],
                    ftb[:, col : col + 512],
                    start=True,
                    stop=False,
                )
            for s in range(4):
                col = c0 * MCH + s * 512
                nc.tensor.matmul(
                    P[:, s * 512 : (s + 1) * 512],
                    XL[:, rs],
                    X[:, col : col + 512],
                    start=False,
                    stop=True,
                )

        # ---- phase W: window pair (cols 0:2048) per row block.  One wide
        # ACT drain (cheaper per element than two 1024 drains) + immediate
        # DVE fold so downstream deps stay fine-grained. ----
        tw = [None] * RB
        for rb in range(RB):
            if wide:
                wlo, whi = 0, 2048
            else:
                wlo, whi = rb * 128 + 64, rb * 128 + 576
            P = psum.tile([128, 2048], f32, tag="P")
            mm_pair(P, rb, 0)
            nc.vector.tensor_reduce(
                posacc[:, rb : rb + 1], P[:, wlo:whi], axis=AX, op=Alu.max
            )
            BW = bwpool.tile([128, 2048], bf16, tag="BW")
            nc.scalar.copy(BW, P)
            t = twpool.tile([128, MCH], bf16, tag="TW")
            nc.vector.tensor_tensor(t, BW[:, 0:MCH], BW[:, MCH:], op=Alu.min)
            tw[rb] = t

        # ---- phase F: far pairs (c2c3, c4c5, c6c7) per row block ----
        for rb in range(RB):
            PA = psum.tile([128, 2048], f32, tag="P")
            mm_pair(PA, rb, 2)
            BA = bfpool.tile([128, 2048], bf16, tag="BA")
            nc.scalar.copy(BA, PA)
            fA = fpool.tile([128, MCH], bf16, tag="FA")
            nc.vector.tensor_tensor(fA, BA[:, 0:MCH], BA[:, MCH:], op=Alu.min)
            PB = psum.tile([128, 2048], f32, tag="P")
            mm_pair(PB, rb, 4)
            BB = bfpool.tile([128, 2048], bf16, tag="BB")
            nc.scalar.copy(BB, PB)
            fB = fpool.tile([128, MCH], bf16, tag="FB")
            nc.vector.tensor_tensor(fB, BB[:, 0:MCH], BB[:, MCH:], op=Alu.min)
            PC = psum.tile([128, 2048], f32, tag="P")
            mm_pair(PC, rb, 6)
            B6 = b6pool.tile([128, MCH], bf16, tag="B6")
            nc.scalar.copy(B6, PC[:, 0:MCH])
            nc.vector.tensor_reduce(
                negacc[:, rb, 0:1], PC[:, MCH:], axis=AX, op=Alu.min
            )
            # DVE min tree over {fA, fB, B6, tw}
            t0 = tpool.tile([128, MCH], bf16, tag="T0")
            nc.vector.tensor_tensor(t0, fA, fB, op=Alu.min)
            t1 = tpool.tile([128, MCH], bf16, tag="T1")
            nc.vector.tensor_tensor(t1, t0, B6, op=Alu.min)
            t2 = tpool.tile([128, MCH], bf16, tag="T2")
            nc.vector.tensor_tensor(t2, t1, tw[rb], op=Alu.min)
            nc.vector.tensor_reduce(negacc[:, rb, 1:2], t2, axis=AX, op=Alu.min)

        # ---- DMA accumulators out; host does bias/sqrt/relu/mean ----
        nc.sync.dma_start(negout.ap(), negacc)
        nc.sync.dma_start(posout.ap(), posacc)

    nc.compile()
    return nc


def _prep_inputs(feature, identity):
    f = np.ascontiguousarray(np.asarray(feature), dtype=np.float32)
    ids = np.asarray(identity).astype(np.int32)
    assert f.shape == (N, D) and ids.shape == (N,)

    perm = np.argsort(ids, kind="stable")
    fs = f[perm]
    ids_s = ids[perm]
    maxcnt = int(np.bincount(ids_s, minlength=NID).max())
    if maxcnt <= 192:
        wide = False
    elif maxcnt <= 256:
        wide = True
    else:
        raise ValueError(f"identity group of {maxcnt} exceeds pos-window margin")

    sq = (fs.astype(np.float64) ** 2).sum(axis=1)
    delta = (sq - S0).astype(np.float32)
    gids = np.arange(NID, dtype=np.int32)

    in_maps = []
    for k in range(NCORES):
        off = (k * RPC - ROW0) % N
        order = (np.arange(N) + off) % N  # local col j <- sorted row order[j]
        idr = ids_s[order]
        dr = delta[order]
        ftb = np.ascontiguousarray(fs[order].T.astype(BF16))  # [128, N]
        onehot = idr[None, :] == gids[:, None]  # [64, N]
        X = np.concatenate(
            [
                np.where(onehot, dr[None, :], 0.0),
                np.where(onehot, 64.0, 0.0),
            ],
            axis=0,
        ).astype(BF16)
        own = slice(ROW0, ROW0 + RPC)
        ftm2 = np.ascontiguousarray((-2.0 * fs[order[own]].T).astype(BF16))
        oh_own = onehot[:, own]
        XL = np.concatenate(
            [np.ones((NID, RPC), np.float32), np.where(oh_own, 64.0, 0.0)],
            axis=0,
        ).astype(BF16)
        in_maps.append(
            {
                "ftb": ftb,
                "xmat": np.ascontiguousarray(X),
                "ftm2": ftm2,
                "xl": np.ascontiguousarray(XL),
            }
        )
    sq_s = sq.astype(np.float32)  # per sorted row
    return in_maps, wide, sq_s


def get_nc(wide):
    key = ("nc", wide)
    if key not in _cache:
        _cache[key] = _build_nc(wide)
    return _cache[key]


def run(feature, identity, **spmd_kwargs):
    from concourse.bass_utils import run_bass_kernel_spmd

    in_maps, wide, sq_s = _prep_inputs(feature, identity)
    nc = get_nc(wide)
    br = run_bass_kernel_spmd(nc, in_maps, core_ids=list(range(NCORES)), **spmd_kwargs)

    terms = []
    for k, r in enumerate(br.results):
        neg = r["negout"].reshape(128, RB, 2).min(axis=2)  # [p, rb]
        pos = r["posout"].reshape(128, RB)
        # local row t = rb*128 + p  <->  sorted row k*RPC + t
        t = np.arange(RPC)
        sqr = sq_s[k * RPC + t].reshape(RB, 128).T  # [p, rb]
        pos_d2 = pos + sqr + S0 - BIG
        neg_d2 = neg + sqr + S0
        pos_d = np.sqrt(np.maximum(pos_d2, 0.0))
        neg_d = np.sqrt(np.maximum(neg_d2, 0.0))
        terms.append(np.maximum(MARGIN + pos_d - neg_d, 0.0))
    loss = np.float32(np.mean(np.stack(terms)))
    return np.asarray(loss), br


def kernel(feature, identity):
    out, _ = run(feature, identity)
    return out


# revision 26
# speedup vs baseline: 1.1434x; 1.1434x over previous
"""BatchHardTripletLoss on 8 Trainium2 NeuronCores (Bass/Tile).

Math: for the n x n squared-distance matrix d2[i,j] = sq_i + sq_j - 2*f_i.f_j,
hardest positive = max_{id_j==id_i} dist, hardest negative = min_{id_j!=id_i},
loss = mean(relu(margin + pos - neg)).  Both extremes commute with sqrt/+sq_i,
so each core reduces P[i,j] = delta_j - 2*G[i,j] + BIG*same[i,j] where
delta_j = sq_j - S0, then the host adds sq_i + S0 back.  The BIG*same and
delta_j terms ride a single K=128 one-hot matmul accumulated on top of the
Gram matmul (BIG = 64*64), so no elementwise masking pass is ever needed.

Sharding: rows are sorted by identity on the host; core k owns sorted rows
[k*1024,(k+1)*1024).  Each core receives the full feature matrix rotated so
its own rows sit at local columns [256,1280) - identity groups are contiguous
after the sort, so every same-id entry of every local row lives in local
columns [0,2048) and the hardest-positive max only scans that window.

All O(n*d) prep (transpose, norms, one-hot operands, bf16 casts) happens on
the host; the device does only DMA-in, the O(n^2) matmuls + reductions, and
DMA-out of per-(partition,row-block) accumulators.  Final sqrt/relu/mean is
host fp32.

Per row block rb (128 rows), columns in 8 chunks of 1024:
  - chunks 0,1 (window): DVE takes the fp32 pos-window max straight off
    psum, ACT drains to bf16 for the hardest-neg min tree.
  - chunks 2..6: ACT drains to bf16; DVE runs the pairwise-min tree at 2x.
  - chunk 7: DVE exact min reduce straight off psum.
(GpSimd can neither touch PSUM nor run tensor_tensor min, so the min path
is split ACT/DVE only.)  Window work for all rbs is emitted first so the PE
can start as soon as the first quarter of the columns has landed.
"""

import numpy as np
import ml_dtypes

BF16 = ml_dtypes.bfloat16

N = 8192
D = 128
NCORES = 8
RPC = N // NCORES  # rows per core
RB = RPC // 128  # row blocks per core
MCH = 1024  # psum chunk (2 banks)
NMC = N // MCH
ROW0 = 256  # local column offset of a core's own rows
BIG = 4096.0
S0 = 128.0
MARGIN = 0.2
NID = 64

_cache = {}


def _build_nc(wide):
    from contextlib import ExitStack

    import concourse.bass as bass
    import concourse.bacc as bacc
    import concourse.mybir as mybir
    import concourse.tile as tile

    f32 = mybir.dt.float32
    bf16 = mybir.dt.bfloat16
    AX = mybir.AxisListType.X
    Alu = mybir.AluOpType

    nc = bacc.Bacc(trn_type="TRN2", target_bir_lowering=False, debug=False)
    # device inputs, all host-prepared (bf16, laid out partition-major so
    # every partition's data is one contiguous DRAM run)
    ftb_d = nc.dram_tensor("ftb", [128, N], bf16, kind="ExternalInput")  # F^T
    x_d = nc.dram_tensor("xmat", [128, N], bf16, kind="ExternalInput")  # extras moving
    ftm2_d = nc.dram_tensor("ftm2", [128, RPC], bf16, kind="ExternalInput")  # -2 F^T own
    xl_d = nc.dram_tensor("xl", [128, RPC], bf16, kind="ExternalInput")  # extras lhsT
    negout = nc.dram_tensor("negout", [128, RB * 2], f32, kind="ExternalOutput")
    posout = nc.dram_tensor("posout", [128, RB * 2], f32, kind="ExternalOutput")

    with ExitStack() as ctx:
        tc = ctx.enter_context(tile.TileContext(nc))
        singles = ctx.enter_context(tc.tile_pool(name="singles", bufs=1))
        psum = ctx.enter_context(tc.tile_pool(name="psum", bufs=4, space="PSUM"))
        bwpool = ctx.enter_context(tc.tile_pool(name="bwpool", bufs=8))
        bfpool = ctx.enter_context(tc.tile_pool(name="bfpool", bufs=2))
        tpool = ctx.enter_context(tc.tile_pool(name="tpool", bufs=6))
        twpool = ctx.enter_context(tc.tile_pool(name="twpool", bufs=RB))

        ftb = singles.tile([128, N], bf16)
        X = singles.tile([128, N], bf16)
        ftm2 = singles.tile([128, RPC], bf16)
        XL = singles.tile([128, RPC], bf16)
        negacc = singles.tile([128, RB, 2], f32)
        posacc = singles.tile([128, RB, 2], f32)
        nc.vector.memset(posacc, -1e9)

        # ---- DMAs: the Gram stream (ftb) gets the sync HW queue to itself
        # so matmuls never wait behind X pieces; X rides the gpsimd queue,
        # which demand-paces against the extras matmuls without hurting ----
        nc.gpsimd.dma_start(ftm2, ftm2_d.ap())
        nc.gpsimd.dma_start(XL, xl_d.ap())
        for g in range(8):
            cs = slice(g * 1024, (g + 1) * 1024)
            nc.sync.dma_start(ftb[:, cs], ftb_d.ap()[:, cs])
            nc.gpsimd.dma_start(X[:, cs], x_d.ap()[:, cs])

        # ---- PE warm-up: a few dummy matmuls gated only on the tiny first
        # DMA, so the HAM clock-gate is at 8/8 when the real stream starts ----
        Pwarm = psum.tile([128, MCH], f32, tag="P")
        for i in range(6):
            nc.tensor.matmul(
                Pwarm[:, :256], ftm2[:, 0:128], ftm2[:, 0:256],
                start=True, stop=True,
            )

        def mm_chunk(P, rb, c):
            # grouped by stationary operand so LDWEIGHTS amortizes
            rs = slice(rb * 128, (rb + 1) * 128)
            for s in range(2):
                col = c * MCH + s * 512
                nc.tensor.matmul(
                    P[:, s * 512 : (s + 1) * 512],
                    ftm2[:, rs],
                    ftb[:, col : col + 512],
                    start=True,
                    stop=False,
                )
            for s in range(2):
                col = c * MCH + s * 512
                nc.tensor.matmul(
                    P[:, s * 512 : (s + 1) * 512],
                    XL[:, rs],
                    X[:, col : col + 512],
                    start=False,
                    stop=True,
                )

        # ---- phase W: window chunks (cols 0:2048) for every row block ----
        # pos-window slices per rb: narrow = [rb*128+64, rb*128+576)
        tw = [None] * RB
        for rb in range(RB):
            if wide:
                wlo, whi = 0, 2048
            else:
                wlo, whi = rb * 128 + 64, rb * 128 + 576
            Bw = []
            for c in range(2):
                P = psum.tile([128, MCH], f32, tag="P")
                mm_chunk(P, rb, c)
                lo = max(wlo, c * MCH) - c * MCH
                hi = min(whi, (c + 1) * MCH) - c * MCH
                if lo < hi:
                    nc.vector.tensor_reduce(
                        posacc[:, rb, c : c + 1], P[:, lo:hi], axis=AX, op=Alu.max
                    )
                B = bwpool.tile([128, MCH], bf16, tag="BW")
                nc.scalar.copy(B, P)
                Bw.append(B)
            t = twpool.tile([128, MCH], bf16, tag="TW")
            nc.vector.tensor_tensor(t, Bw[0], Bw[1], op=Alu.min)
            tw[rb] = t

        # ---- phase F: far chunks 2..7 per row block.  Chunks 2..5 drain
        # into slices of ONE contiguous tile so the first min-tree level is
        # a single wide 2x tensor_tensor over two 2048 slices. ----
        for rb in range(RB):
            Brow = bfpool.tile([128, 4, MCH], bf16, tag="BROW")
            for i, c in enumerate(range(2, 6)):
                P = psum.tile([128, MCH], f32, tag="P")
                mm_chunk(P, rb, c)
                nc.scalar.copy(Brow[:, i, :], P)
            P6 = psum.tile([128, MCH], f32, tag="P")
            mm_chunk(P6, rb, 6)
            B6 = tpool.tile([128, MCH], bf16, tag="B6")
            nc.scalar.copy(B6, P6)
            P = psum.tile([128, MCH], f32, tag="P")
            mm_chunk(P, rb, 7)
            nc.vector.tensor_reduce(negacc[:, rb, 0:1], P, axis=AX, op=Alu.min)
            # y = min(B6, tw) runs early; wide m01 covers B2..B5 in one op
            y = tpool.tile([128, MCH], bf16, tag="Y")
            nc.vector.tensor_tensor(y, B6, tw[rb], op=Alu.min)
            m01 = tpool.tile([128, 2 * MCH], bf16, tag="M01")
            nc.vector.tensor_tensor(
                m01, Brow[:, 0:2, :], Brow[:, 2:4, :], op=Alu.min
            )
            z = tpool.tile([128, MCH], bf16, tag="Z")
            nc.vector.tensor_tensor(z, m01[:, 0:MCH], m01[:, MCH:], op=Alu.min)
            w = tpool.tile([128, MCH], bf16, tag="W")
            nc.vector.tensor_tensor(w, z, y, op=Alu.min)
            nc.vector.tensor_reduce(negacc[:, rb, 1:2], w, axis=AX, op=Alu.min)

        # ---- DMA accumulators out; host does bias/sqrt/relu/mean ----
        nc.sync.dma_start(negout.ap(), negacc)
        nc.sync.dma_start(posout.ap(), posacc)

    nc.compile()
    return nc


def _prep_inputs(feature, identity):
    f = np.ascontiguousarray(np.asarray(feature), dtype=np.float32)
    ids = np.asarray(identity).astype(np.int32)
    assert f.shape == (N, D) and ids.shape == (N,)

    perm = np.argsort(ids, kind="stable")
    fs = f[perm]
    ids_s = ids[perm]
    maxcnt = int(np.bincount(ids_s, minlength=NID).max())
    if maxcnt <= 192:
        wide = False
    elif maxcnt <= 256:
        wide = True
    else:
        raise ValueError(f"identity group of {maxcnt} exceeds pos-window margin")

    sq = (fs.astype(np.float64) ** 2).sum(axis=1)
    delta = (sq - S0).astype(np.float32)
    gids = np.arange(NID, dtype=np.int32)

    in_maps = []
    for k in range(NCORES):
        off = (k * RPC - ROW0) % N
        order = (np.arange(N) + off) % N  # local col j <- sorted row order[j]
        idr = ids_s[order]
        dr = delta[order]
        ftb = np.ascontiguousarray(fs[order].T.astype(BF16))  # [128, N]
        onehot = idr[None, :] == gids[:, None]  # [64, N]
        X = np.concatenate(
            [
                np.where(onehot, dr[None, :], 0.0),
                np.where(onehot, 64.0, 0.0),
            ],
            axis=0,
        ).astype(BF16)
        own = slice(ROW0, ROW0 + RPC)
        ftm2 = np.ascontiguousarray((-2.0 * fs[order[own]].T).astype(BF16))
        oh_own = onehot[:, own]
        XL = np.concatenate(
            [np.ones((NID, RPC), np.float32), np.where(oh_own, 64.0, 0.0)],
            axis=0,
        ).astype(BF16)
        in_maps.append(
            {
                "ftb": ftb,
                "xmat": np.ascontiguousarray(X),
                "ftm2": ftm2,
                "xl": np.ascontiguousarray(XL),
            }
        )
    sq_s = sq.astype(np.float32)  # per sorted row
    return in_maps, wide, sq_s


def get_nc(wide):
    key = ("nc", wide)
    if key not in _cache:
        _cache[key] = _build_nc(wide)
    return _cache[key]


def run(feature, identity, **spmd_kwargs):
    from concourse.bass_utils import run_bass_kernel_spmd

    in_maps, wide, sq_s = _prep_inputs(feature, identity)
    nc = get_nc(wide)
    br = run_bass_kernel_spmd(nc, in_maps, core_ids=list(range(NCORES)), **spmd_kwargs)

    terms = []
    for k, r in enumerate(br.results):
        neg = r["negout"].reshape(128, RB, 2).min(axis=2)  # [p, rb]
        pos = r["posout"].reshape(128, RB, 2).max(axis=2)
        # local row t = rb*128 + p  <->  sorted row k*RPC + t
        t = np.arange(RPC)
        sqr = sq_s[k * RPC + t].reshape(RB, 128).T  # [p, rb]
        pos_d2 = pos + sqr + S0 - BIG
        neg_d2 = neg + sqr + S0
        pos_d = np.sqrt(np.maximum(pos_d2, 0.0))
        neg_d = np.sqrt(np.maximum(neg_d2, 0.0))
        terms.append(np.maximum(MARGIN + pos_d - neg_d, 0.0))
    loss = np.float32(np.mean(np.stack(terms)))
    return np.asarray(loss), br


def kernel(feature, identity):
    out, _ = run(feature, identity)
    return out


# revision 27
# speedup vs baseline: 1.1731x; 1.0259x over previous
"""BatchHardTripletLoss on 8 Trainium2 NeuronCores (Bass/Tile).

Math: for the n x n squared-distance matrix d2[i,j] = sq_i + sq_j - 2*f_i.f_j,
hardest positive = max_{id_j==id_i} dist, hardest negative = min_{id_j!=id_i},
loss = mean(relu(margin + pos - neg)).  Both extremes commute with sqrt/+sq_i,
so each core reduces P[i,j] = delta_j - 2*G[i,j] + BIG*same[i,j] where
delta_j = sq_j - S0, then the host adds sq_i + S0 back.  The BIG*same and
delta_j terms ride a single K=128 one-hot matmul accumulated on top of the
Gram matmul (BIG = 64*64), so no elementwise masking pass is ever needed.

Sharding: rows are sorted by identity on the host; core k owns sorted rows
[k*1024,(k+1)*1024).  Each core receives the full feature matrix rotated so
its own rows sit at local columns [256,1280) - identity groups are contiguous
after the sort, so every same-id entry of every local row lives in local
columns [0,2048) and the hardest-positive max only scans that window.

All O(n*d) prep (transpose, norms, one-hot operands, bf16 casts) happens on
the host; the device does only DMA-in, the O(n^2) matmuls + reductions, and
DMA-out of per-(partition,row-block) accumulators.  Final sqrt/relu/mean is
host fp32.

Per row block rb (128 rows), columns in 8 chunks of 1024:
  - chunks 0,1 (window): DVE takes the fp32 pos-window max straight off
    psum, ACT drains to bf16 for the hardest-neg min tree.
  - chunks 2..6: ACT drains to bf16; DVE runs the pairwise-min tree at 2x.
  - chunk 7: DVE exact min reduce straight off psum.
(GpSimd can neither touch PSUM nor run tensor_tensor min, so the min path
is split ACT/DVE only.)  Window work for all rbs is emitted first so the PE
can start as soon as the first quarter of the columns has landed.
"""

import numpy as np
import ml_dtypes

BF16 = ml_dtypes.bfloat16

N = 8192
D = 128
NCORES = 8
RPC = N // NCORES  # rows per core
RB = RPC // 128  # row blocks per core
MCH = 1024  # psum chunk (2 banks)
NMC = N // MCH
ROW0 = 256  # local column offset of a core's own rows
BIG = 4096.0
S0 = 128.0
MARGIN = 0.2
NID = 64

_cache = {}


def _build_nc(wide):
    from contextlib import ExitStack

    import concourse.bass as bass
    import concourse.bacc as bacc
    import concourse.mybir as mybir
    import concourse.tile as tile

    f32 = mybir.dt.float32
    bf16 = mybir.dt.bfloat16
    AX = mybir.AxisListType.X
    Alu = mybir.AluOpType

    nc = bacc.Bacc(trn_type="TRN2", target_bir_lowering=False, debug=False)
    # device inputs, all host-prepared (bf16, laid out partition-major so
    # every partition's data is one contiguous DRAM run)
    ftb_d = nc.dram_tensor("ftb", [128, N], bf16, kind="ExternalInput")  # F^T
    x_d = nc.dram_tensor("xmat", [128, N], bf16, kind="ExternalInput")  # extras moving
    ftm2_d = nc.dram_tensor("ftm2", [128, RPC], bf16, kind="ExternalInput")  # -2 F^T own
    xl_d = nc.dram_tensor("xl", [128, RPC], bf16, kind="ExternalInput")  # extras lhsT
    negout = nc.dram_tensor("negout", [128, RB * 2], f32, kind="ExternalOutput")
    posout = nc.dram_tensor("posout", [128, RB * 2], f32, kind="ExternalOutput")

    with ExitStack() as ctx:
        tc = ctx.enter_context(tile.TileContext(nc))
        singles = ctx.enter_context(tc.tile_pool(name="singles", bufs=1))
        psum = ctx.enter_context(tc.tile_pool(name="psum", bufs=4, space="PSUM"))
        bwpool = ctx.enter_context(tc.tile_pool(name="bwpool", bufs=8))
        bfpool = ctx.enter_context(tc.tile_pool(name="bfpool", bufs=6))
        tpool = ctx.enter_context(tc.tile_pool(name="tpool", bufs=6))
        twpool = ctx.enter_context(tc.tile_pool(name="twpool", bufs=RB))

        ftb = singles.tile([128, N], bf16)
        X = singles.tile([128, N], bf16)
        ftm2 = singles.tile([128, RPC], bf16)
        XL = singles.tile([128, RPC], bf16)
        negacc = singles.tile([128, RB, 2], f32)
        posacc = singles.tile([128, RB, 2], f32)
        nc.vector.memset(posacc, -1e9)

        # ---- DMAs: the Gram stream (ftb) gets the sync HW queue to itself
        # so matmuls never wait behind X pieces; X rides the gpsimd queue,
        # which demand-paces against the extras matmuls without hurting ----
        nc.gpsimd.dma_start(ftm2, ftm2_d.ap())
        nc.gpsimd.dma_start(XL, xl_d.ap())
        for g in range(8):
            cs = slice(g * 1024, (g + 1) * 1024)
            nc.sync.dma_start(ftb[:, cs], ftb_d.ap()[:, cs])
            nc.gpsimd.dma_start(X[:, cs], x_d.ap()[:, cs])

        # ---- PE warm-up: a few dummy matmuls gated only on the tiny first
        # DMA, so the HAM clock-gate is at 8/8 when the real stream starts ----
        Pwarm = psum.tile([128, MCH], f32, tag="P")
        for i in range(6):
            nc.tensor.matmul(
                Pwarm[:, :256], ftm2[:, 0:128], ftm2[:, 0:256],
                start=True, stop=True,
            )

        def mm_chunk(P, rb, c):
            # grouped by stationary operand so LDWEIGHTS amortizes
            rs = slice(rb * 128, (rb + 1) * 128)
            for s in range(2):
                col = c * MCH + s * 512
                nc.tensor.matmul(
                    P[:, s * 512 : (s + 1) * 512],
                    ftm2[:, rs],
                    ftb[:, col : col + 512],
                    start=True,
                    stop=False,
                )
            for s in range(2):
                col = c * MCH + s * 512
                nc.tensor.matmul(
                    P[:, s * 512 : (s + 1) * 512],
                    XL[:, rs],
                    X[:, col : col + 512],
                    start=False,
                    stop=True,
                )

        # ---- phase W: window chunks (cols 0:2048) for every row block ----
        # pos-window slices per rb: narrow = [rb*128+64, rb*128+576)
        tw = [None] * RB
        for rb in range(RB):
            if wide:
                wlo, whi = 0, 2048
            else:
                wlo, whi = rb * 128 + 64, rb * 128 + 576
            Bw = []
            for c in range(2):
                P = psum.tile([128, MCH], f32, tag="P")
                mm_chunk(P, rb, c)
                lo = max(wlo, c * MCH) - c * MCH
                hi = min(whi, (c + 1) * MCH) - c * MCH
                if lo < hi:
                    nc.vector.tensor_reduce(
                        posacc[:, rb, c : c + 1], P[:, lo:hi], axis=AX, op=Alu.max
                    )
                B = bwpool.tile([128, MCH], bf16, tag="BW")
                nc.scalar.copy(B, P)
                Bw.append(B)
            t = twpool.tile([128, MCH], bf16, tag="TW")
            nc.vector.tensor_tensor(t, Bw[0], Bw[1], op=Alu.min)
            tw[rb] = t

        # ---- phase F: far chunks 2..7 per row block ----
        for rb in range(RB):
            Bs = []
            for c in range(2, 7):
                P = psum.tile([128, MCH], f32, tag="P")
                mm_chunk(P, rb, c)
                B = bfpool.tile([128, MCH], bf16, tag="BF")
                nc.scalar.copy(B, P)
                Bs.append(B)
            P = psum.tile([128, MCH], f32, tag="P")
            mm_chunk(P, rb, 7)
            nc.vector.tensor_reduce(negacc[:, rb, 0:1], P, axis=AX, op=Alu.min)
            # DVE min tree over {B2..B6, tw}
            u0 = tpool.tile([128, MCH], bf16, tag="U0")
            nc.vector.tensor_tensor(u0, Bs[0], Bs[1], op=Alu.min)
            u1 = tpool.tile([128, MCH], bf16, tag="U1")
            nc.vector.tensor_tensor(u1, Bs[2], Bs[3], op=Alu.min)
            u2 = tpool.tile([128, MCH], bf16, tag="U2")
            nc.vector.tensor_tensor(u2, u0, u1, op=Alu.min)
            u3 = tpool.tile([128, MCH], bf16, tag="U3")
            nc.vector.tensor_tensor(u3, Bs[4], tw[rb], op=Alu.min)
            u4 = tpool.tile([128, MCH], bf16, tag="U4")
            nc.vector.tensor_tensor(u4, u2, u3, op=Alu.min)
            nc.vector.tensor_reduce(negacc[:, rb, 1:2], u4, axis=AX, op=Alu.min)

        # ---- DMA accumulators out; host does bias/sqrt/relu/mean ----
        nc.sync.dma_start(negout.ap(), negacc)
        nc.sync.dma_start(posout.ap(), posacc)

    nc.compile()
    return nc


def _prep_inputs(feature, identity):
    f = np.ascontiguousarray(np.asarray(feature), dtype=np.float32)
    ids = np.asarray(identity).astype(np.int32)
    assert f.shape == (N, D) and ids.shape == (N,)

    perm = np.argsort(ids, kind="stable")
    fs = f[perm]
    ids_s = ids[perm]
    maxcnt = int(np.bincount(ids_s, minlength=NID).max())
    if maxcnt <= 192:
        wide = False
    elif maxcnt <= 256:
        wide = True
    else:
        raise ValueError(f"identity group of {maxcnt} exceeds pos-window margin")

    sq = (fs.astype(np.float64) ** 2).sum(axis=1)
    delta = (sq - S0).astype(np.float32)
    gids = np.arange(NID, dtype=np.int32)

    in_maps = []
    for k in range(NCORES):
        off = (k * RPC - ROW0) % N
        order = (np.arange(N) + off) % N  # local col j <- sorted row order[j]
        idr = ids_s[order]
        dr = delta[order]
        ftb = np.ascontiguousarray(fs[order].T.astype(BF16))  # [128, N]
        onehot = idr[None, :] == gids[:, None]  # [64, N]
        X = np.concatenate(
            [
                np.where(onehot, dr[None, :], 0.0),
                np.where(onehot, 64.0, 0.0),
            ],
            axis=0,
        ).astype(BF16)
        own = slice(ROW0, ROW0 + RPC)
        ftm2 = np.ascontiguousarray((-2.0 * fs[order[own]].T).astype(BF16))
        oh_own = onehot[:, own]
        XL = np.concatenate(
            [np.ones((NID, RPC), np.float32), np.where(oh_own, 64.0, 0.0)],
            axis=0,
        ).astype(BF16)
        in_maps.append(
            {
                "ftb": ftb,
                "xmat": np.ascontiguousarray(X),
                "ftm2": ftm2,
                "xl": np.ascontiguousarray(XL),
            }
        )
    sq_s = sq.astype(np.float32)  # per sorted row
    return in_maps, wide, sq_s


def get_nc(wide):
    key = ("nc", wide)
    if key not in _cache:
        _cache[key] = _build_nc(wide)
    return _cache[key]


def run(feature, identity, **spmd_kwargs):
    from concourse.bass_utils import run_bass_kernel_spmd

    in_maps, wide, sq_s = _prep_inputs(feature, identity)
    nc = get_nc(wide)
    br = run_bass_kernel_spmd(nc, in_maps, core_ids=list(range(NCORES)), **spmd_kwargs)

    terms = []
    for k, r in enumerate(br.results):
        neg = r["negout"].reshape(128, RB, 2).min(axis=2)  # [p, rb]
        pos = r["posout"].reshape(128, RB, 2).max(axis=2)
        # local row t = rb*128 + p  <->  sorted row k*RPC + t
        t = np.arange(RPC)
        sqr = sq_s[k * RPC + t].reshape(RB, 128).T  # [p, rb]
        pos_d2 = pos + sqr + S0 - BIG
        neg_d2 = neg + sqr + S0
        pos_d = np.sqrt(np.maximum(pos_d2, 0.0))
        neg_d = np.sqrt(np.maximum(neg_d2, 0.0))
        terms.append(np.maximum(MARGIN + pos_d - neg_d, 0.0))
    loss = np.float32(np.mean(np.stack(terms)))
    return np.asarray(loss), br


def kernel(feature, identity):
    out, _ = run(feature, identity)
    return out


# revision 28
# speedup vs baseline: 1.1766x; 1.0030x over previous
"""BatchHardTripletLoss on 8 Trainium2 NeuronCores (Bass/Tile).

Math: for the n x n squared-distance matrix d2[i,j] = sq_i + sq_j - 2*f_i.f_j,
hardest positive = max_{id_j==id_i} dist, hardest negative = min_{id_j!=id_i},
loss = mean(relu(margin + pos - neg)).  Both extremes commute with sqrt/+sq_i,
so each core reduces P[i,j] = delta_j - 2*G[i,j] + BIG*same[i,j] where
delta_j = sq_j - S0, then the host adds sq_i + S0 back.  The BIG*same and
delta_j terms ride a single K=128 one-hot matmul accumulated on top of the
Gram matmul (BIG = 64*64), so no elementwise masking pass is ever needed.

Sharding: rows are sorted by identity on the host; core k owns sorted rows
[k*1024,(k+1)*1024).  Each core receives the full feature matrix rotated so
its own rows sit at local columns [256,1280) - identity groups are contiguous
after the sort, so every same-id entry of every local row lives in local
columns [0,2048) and the hardest-positive max only scans that window.

All O(n*d) prep (transpose, norms, one-hot operands, bf16 casts) happens on
the host; the device does only DMA-in, the O(n^2) matmuls + reductions, and
DMA-out of per-(partition,row-block) accumulators.  Final sqrt/relu/mean is
host fp32.

Per row block rb (128 rows), columns in 8 chunks of 1024:
  - chunks 0,1 (window): DVE takes the fp32 pos-window max straight off
    psum, ACT drains to bf16 for the hardest-neg min tree.
  - chunks 2..6: ACT drains to bf16; DVE runs the pairwise-min tree at 2x.
  - chunk 7: DVE exact min reduce straight off psum.
(GpSimd can neither touch PSUM nor run tensor_tensor min, so the min path
is split ACT/DVE only.)  Window work for all rbs is emitted first so the PE
can start as soon as the first quarter of the columns has landed.
"""

import numpy as np
import ml_dtypes

BF16 = ml_dtypes.bfloat16

N = 8192
D = 128
NCORES = 8
RPC = N // NCORES  # rows per core
RB = RPC // 128  # row blocks per core
MCH = 1024  # psum chunk (2 banks)
NMC = N // MCH
ROW0 = 256  # local column offset of a core's own rows
BIG = 4096.0
S0 = 128.0
MARGIN = 0.2
NID = 64

_cache = {}


def _build_nc(wide):
    from contextlib import ExitStack

    import concourse.bass as bass
    import concourse.bacc as bacc
    import concourse.mybir as mybir
    import concourse.tile as tile

    f32 = mybir.dt.float32
    bf16 = mybir.dt.bfloat16
    AX = mybir.AxisListType.X
    Alu = mybir.AluOpType

    nc = bacc.Bacc(trn_type="TRN2", target_bir_lowering=False, debug=False)
    # device inputs, all host-prepared (bf16, laid out partition-major so
    # every partition's data is one contiguous DRAM run)
    ftb_d = nc.dram_tensor("ftb", [128, N], bf16, kind="ExternalInput")  # F^T
    x_d = nc.dram_tensor("xmat", [128, N], bf16, kind="ExternalInput")  # extras moving
    ftm2_d = nc.dram_tensor("ftm2", [128, RPC], bf16, kind="ExternalInput")  # -2 F^T own
    xl_d = nc.dram_tensor("xl", [128, RPC], bf16, kind="ExternalInput")  # extras lhsT
    negout = nc.dram_tensor("negout", [128, RB * 2], f32, kind="ExternalOutput")
    posout = nc.dram_tensor("posout", [128, RB * 2], f32, kind="ExternalOutput")

    with ExitStack() as ctx:
        tc = ctx.enter_context(tile.TileContext(nc))
        singles = ctx.enter_context(tc.tile_pool(name="singles", bufs=1))
        psum = ctx.enter_context(tc.tile_pool(name="psum", bufs=4, space="PSUM"))
        bwpool = ctx.enter_context(tc.tile_pool(name="bwpool", bufs=8))
        bfpool = ctx.enter_context(tc.tile_pool(name="bfpool", bufs=6))
        tpool = ctx.enter_context(tc.tile_pool(name="tpool", bufs=6))
        twpool = ctx.enter_context(tc.tile_pool(name="twpool", bufs=RB))

        ftb = singles.tile([128, N], bf16)
        X = singles.tile([128, N], bf16)
        ftm2 = singles.tile([128, RPC], bf16)
        XL = singles.tile([128, RPC], bf16)
        negacc = singles.tile([128, RB, 2], f32)
        posacc = singles.tile([128, RB, 2], f32)
        nc.vector.memset(posacc, -1e9)

        # ---- DMAs: the Gram stream (ftb) gets the sync HW queue to itself
        # so matmuls never wait behind X pieces; ftm2/XL/X ride the scalar
        # HW queue, whose issue cost lands in ACT's idle head.  This gets
        # ftm2 on chip ~2.5us earlier than the gpsimd software queue, which
        # pulls the whole warm-up + matmul stream forward. ----
        nc.scalar.dma_start(ftm2, ftm2_d.ap())
        nc.scalar.dma_start(XL, xl_d.ap())
        for g in range(8):
            cs = slice(g * 1024, (g + 1) * 1024)
            nc.sync.dma_start(ftb[:, cs], ftb_d.ap()[:, cs])
            nc.scalar.dma_start(X[:, cs], x_d.ap()[:, cs])

        # ---- PE warm-up: a few dummy matmuls gated only on the tiny first
        # DMA, so the HAM clock-gate is at 8/8 when the real stream starts ----
        Pwarm = psum.tile([128, MCH], f32, tag="P")
        for i in range(6):
            nc.tensor.matmul(
                Pwarm[:, :256], ftm2[:, 0:128], ftm2[:, 0:256],
                start=True, stop=True,
            )

        def mm_chunk(P, rb, c):
            # grouped by stationary operand so LDWEIGHTS amortizes
            rs = slice(rb * 128, (rb + 1) * 128)
            for s in range(2):
                col = c * MCH + s * 512
                nc.tensor.matmul(
                    P[:, s * 512 : (s + 1) * 512],
                    ftm2[:, rs],
                    ftb[:, col : col + 512],
                    start=True,
                    stop=False,
                )
            for s in range(2):
                col = c * MCH + s * 512
                nc.tensor.matmul(
                    P[:, s * 512 : (s + 1) * 512],
                    XL[:, rs],
                    X[:, col : col + 512],
                    start=False,
                    stop=True,
                )

        # ---- phase W: window chunks (cols 0:2048) for every row block ----
        # pos-window slices per rb: narrow = [rb*128+64, rb*128+576)
        tw = [None] * RB
        for rb in range(RB):
            if wide:
                wlo, whi = 0, 2048
            else:
                wlo, whi = rb * 128 + 64, rb * 128 + 576
            Bw = []
            for c in range(2):
                P = psum.tile([128, MCH], f32, tag="P")
                mm_chunk(P, rb, c)
                lo = max(wlo, c * MCH) - c * MCH
                hi = min(whi, (c + 1) * MCH) - c * MCH
                if lo < hi:
                    nc.vector.tensor_reduce(
                        posacc[:, rb, c : c + 1], P[:, lo:hi], axis=AX, op=Alu.max
                    )
                B = bwpool.tile([128, MCH], bf16, tag="BW")
                nc.scalar.copy(B, P)
                Bw.append(B)
            t = twpool.tile([128, MCH], bf16, tag="TW")
            nc.vector.tensor_tensor(t, Bw[0], Bw[1], op=Alu.min)
            tw[rb] = t

        # ---- phase F: far chunks 2..7 per row block ----
        for rb in range(RB):
            Bs = []
            for c in range(2, 7):
                P = psum.tile([128, MCH], f32, tag="P")
                mm_chunk(P, rb, c)
                B = bfpool.tile([128, MCH], bf16, tag="BF")
                nc.scalar.copy(B, P)
                Bs.append(B)
            P = psum.tile([128, MCH], f32, tag="P")
            mm_chunk(P, rb, 7)
            nc.vector.tensor_reduce(negacc[:, rb, 0:1], P, axis=AX, op=Alu.min)
            # DVE min tree over {B2..B6, tw}
            u0 = tpool.tile([128, MCH], bf16, tag="U0")
            nc.vector.tensor_tensor(u0, Bs[0], Bs[1], op=Alu.min)
            u1 = tpool.tile([128, MCH], bf16, tag="U1")
            nc.vector.tensor_tensor(u1, Bs[2], Bs[3], op=Alu.min)
            u2 = tpool.tile([128, MCH], bf16, tag="U2")
            nc.vector.tensor_tensor(u2, u0, u1, op=Alu.min)
            u3 = tpool.tile([128, MCH], bf16, tag="U3")
            nc.vector.tensor_tensor(u3, Bs[4], tw[rb], op=Alu.min)
            u4 = tpool.tile([128, MCH], bf16, tag="U4")
            nc.vector.tensor_tensor(u4, u2, u3, op=Alu.min)
            nc.vector.tensor_reduce(negacc[:, rb, 1:2], u4, axis=AX, op=Alu.min)

        # ---- DMA accumulators out; host does bias/sqrt/relu/mean ----
        nc.sync.dma_start(negout.ap(), negacc)
        nc.sync.dma_start(posout.ap(), posacc)

    nc.compile()
    return nc


def _prep_inputs(feature, identity):
    f = np.ascontiguousarray(np.asarray(feature), dtype=np.float32)
    ids = np.asarray(identity).astype(np.int32)
    assert f.shape == (N, D) and ids.shape == (N,)

    perm = np.argsort(ids, kind="stable")
    fs = f[perm]
    ids_s = ids[perm]
    maxcnt = int(np.bincount(ids_s, minlength=NID).max())
    if maxcnt <= 192:
        wide = False
    elif maxcnt <= 256:
        wide = True
    else:
        raise ValueError(f"identity group of {maxcnt} exceeds pos-window margin")

    sq = (fs.astype(np.float64) ** 2).sum(axis=1)
    delta = (sq - S0).astype(np.float32)
    gids = np.arange(NID, dtype=np.int32)

    in_maps = []
    for k in range(NCORES):
        off = (k * RPC - ROW0) % N
        order = (np.arange(N) + off) % N  # local col j <- sorted row order[j]
        idr = ids_s[order]
        dr = delta[order]
        ftb = np.ascontiguousarray(fs[order].T.astype(BF16))  # [128, N]
        onehot = idr[None, :] == gids[:, None]  # [64, N]
        X = np.concatenate(
            [
                np.where(onehot, dr[None, :], 0.0),
                np.where(onehot, 64.0, 0.0),
            ],
            axis=0,
        ).astype(BF16)
        own = slice(ROW0, ROW0 + RPC)
        ftm2 = np.ascontiguousarray((-2.0 * fs[order[own]].T).astype(BF16))
        oh_own = onehot[:, own]
        XL = np.concatenate(
            [np.ones((NID, RPC), np.float32), np.where(oh_own, 64.0, 0.0)],
            axis=0,
        ).astype(BF16)
        in_maps.append(
            {
                "ftb": ftb,
                "xmat": np.ascontiguousarray(X),
                "ftm2": ftm2,
                "xl": np.ascontiguousarray(XL),
            }
        )
    sq_s = sq.astype(np.float32)  # per sorted row
    return in_maps, wide, sq_s


def get_nc(wide):
    key = ("nc", wide)
    if key not in _cache:
        _cache[key] = _build_nc(wide)
    return _cache[key]


def run(feature, identity, **spmd_kwargs):
    from concourse.bass_utils import run_bass_kernel_spmd

    in_maps, wide, sq_s = _prep_inputs(feature, identity)
    nc = get_nc(wide)
    br = run_bass_kernel_spmd(nc, in_maps, core_ids=list(range(NCORES)), **spmd_kwargs)

    terms = []
    for k, r in enumerate(br.results):
        neg = r["negout"].reshape(128, RB, 2).min(axis=2)  # [p, rb]
        pos = r["posout"].reshape(128, RB, 2).max(axis=2)
        # local row t = rb*128 + p  <->  sorted row k*RPC + t
        t = np.arange(RPC)
        sqr = sq_s[k * RPC + t].reshape(RB, 128).T  # [p, rb]
        pos_d2 = pos + sqr + S0 - BIG
        neg_d2 = neg + sqr + S0
        pos_d = np.sqrt(np.maximum(pos_d2, 0.0))
        neg_d = np.sqrt(np.maximum(neg_d2, 0.0))
        terms.append(np.maximum(MARGIN + pos_d - neg_d, 0.0))
    loss = np.float32(np.mean(np.stack(terms)))
    return np.asarray(loss), br


def kernel(feature, identity):
    out, _ = run(feature, identity)
    return out
